# revision 9
# baseline (speedup 1.0000x reference)
"""Lie-series expansion kernel for Trainium2 (8 NeuronCores, data-parallel).

result = x + sum_{i=1..order} z_i,  z_i = (1/i) * sum_g diag(theta_g) z_{i-1} A_g

Data-parallel over batch (512 rows/core), z kept transposed [feature_part,
batch_free]. Each core SORTS its rows by Theta_b = sum_g theta[g,b]^2
descending. A per-step schedule (n_i, h_i) processes only the first n_i
sorted rows (light rows' series terms decay fast and are truncated):
rows [0,h_i) via fp16 matmuls, rows [h_i,n_i) via fp8 e4m3 DoubleRow
matmuls (2 contraction k-tiles per instruction, 2x PE throughput).

The running result is accumulated in 4 dedicated PSUM banks by scaled
identity matmuls ((i+1)*I against the drained z_i/(i+1)), so the per-step
1/i scale folds into the Act-engine PSUM drain and the DVE W-build
(W_g = theta_g * z) is a plain tensor_tensor that qualifies for 16-bit
2x mode. fp8 W tiles (1x DVE mode) are split between DVE and the Pool
engine.
"""

import numpy as np
import ml_dtypes

import concourse.bass as bass
import concourse.bacc as bacc
import concourse.mybir as mybir
from concourse import tile
from concourse.bass_utils import run_bass_kernel_spmd

G, B, F = 8, 4096, 512
NCORES = 8
BLOC = B // NCORES          # 512 batch rows per core
P = 128                     # partitions
FT = F // P                 # 4 feature tiles
GP = G // 2                 # 4 fp8 generator pairs
NK = G * FT                 # 32 contraction k-tiles per step
F32 = mybir.dt.float32
F16 = mybir.dt.float16
F8 = mybir.dt.float8e4
MULT = mybir.AluOpType.mult
DR = mybir.MatmulPerfMode.DoubleRow

# Per-step (n, h): rows [0,h) fp16, [h,n) fp8, [n,512) truncated.
# Tuned via numpy simulation of the exact quantization (err_study).
SCHED10 = [(512, 512), (512, 512), (512, 128), (512, 128), (448, 128),
           (352, 352), (240, 240), (144, 144), (72, 72), (32, 32)]

# how many of the 8 per-drain fp8 W-build ops go to the Pool engine
POOL_W8_OPS = 3

_cache = {}


def _sched(order):
    if order == 10:
        return SCHED10
    return [(BLOC, BLOC)] * order


def _build(order: int):
    if order in _cache:
        return _cache[order]
    sched = _sched(order)
    d = len(sched)
    use8 = any(h < n for (n, h) in sched)

    nc = bacc.Bacc("TRN2", target_bir_lowering=False, debug=False,
                   num_devices=NCORES)

    # host layouts:
    # A16[p, ((ho*G+g)*FT+fi)*P + m] = A[g, fi*P+p, ho*P+m]     (fp16)
    # A8 [p, (((ho*GP+gp)*FT+fi)*2+j)*P + m] = A[2gp+j, fi*P+p, ho*P+m]
    # th16[p, g*BLOC + b] = theta_sorted[g, b]  (bcast along p)
    # xT16[p, fi*BLOC + b] = x_sorted[b, fi*P+p]
    # I16[p, c*P + m] = (c+1) * (p == m)
    A16_d = nc.dram_tensor("A16", [P, FT * NK * P], F16, kind="ExternalInput")
    th_d = nc.dram_tensor("th", [P, G * BLOC], F16, kind="ExternalInput")
    xT_d = nc.dram_tensor("xT", [P, FT * BLOC], F16, kind="ExternalInput")
    I_d = nc.dram_tensor("I", [P, d * P], F16, kind="ExternalInput")
    if use8:
        A8_d = nc.dram_tensor("A8", [P, FT * NK * P], F8, kind="ExternalInput")
    out_d = nc.dram_tensor("outT", [F, BLOC], F32, kind="ExternalOutput")

    with tile.TileContext(nc) as tc:
        with (
            tc.tile_pool(name="const", bufs=1) as cpool,
            tc.tile_pool(name="z", bufs=2) as zpool,
            tc.tile_pool(name="w", bufs=2) as wpool,
            tc.tile_pool(name="work", bufs=1, space=bass.MemorySpace.PSUM) as ppool,
            tc.tile_pool(name="res", bufs=1, space=bass.MemorySpace.PSUM) as rpool,
        ):
            # ---- DMA feed (2 rings) in consumption order ----
            rings = [nc.sync, nc.scalar, nc.gpsimd]
            rr = [0]

            def dma(dst_ap, src_ap):
                rings[rr[0] % len(rings)].dma_start(dst_ap, src_ap)
                rr[0] += 1

            xT = cpool.tile([P, FT, BLOC], F16, name="xT")
            th = cpool.tile([P, G, BLOC], F16, name="th")
            A16 = cpool.tile([P, FT, G, FT, P], F16, name="A16")
            I16 = cpool.tile([P, d, P], F16, name="I16")
            A8 = cpool.tile([P, FT, GP, FT, 2, P], F8, name="A8") if use8 \
                else None

            CH = G * FT * P   # columns per ho block
            CG = FT * P       # columns per (ho, g) block
            for fi in range(FT):
                dma(xT[:, fi, :], xT_d[:, fi * BLOC:(fi + 1) * BLOC])
            # per-g feed so step-1 bank 0 (g-outer) chases the stream
            for g in range(G):
                dma(th[:, g, :], th_d[:, g * BLOC:(g + 1) * BLOC])
                dma(A16[:, 0, g], A16_d[:, g * CG:(g + 1) * CG])
            for c in range(d):
                dma(I16[:, c, :], I_d[:, c * P:(c + 1) * P])
            for ho in range(1, FT):
                for g in range(G):
                    dma(A16[:, ho, g],
                        A16_d[:, ho * CH + g * CG:ho * CH + (g + 1) * CG])
            if use8:
                for ho in range(FT):
                    dma(A8[:, ho], A8_d[:, ho * CH:(ho + 1) * CH])

            # ---- psum banks: 4 work + 4 result accumulators ----
            work = [ppool.tile([P, BLOC], F32, tag=f"ps{ho}", name=f"ps{ho}")
                    for ho in range(FT)]
            resb = [rpool.tile([P, BLOC], F32, tag=f"res{ho}", name=f"res{ho}")
                    for ho in range(FT)]

            def w16_alloc(i):
                return wpool.tile([P, G, FT, BLOC], F16, tag="w16",
                                  name=f"w16_{i}")

            def w8_alloc(i):
                return wpool.tile([P, GP, FT, 2, BLOC], F8, tag="w8",
                                  name=f"w8_{i}")

            def build_w_chunk(W16n, W8tn, zsrc, fi, hn, widn):
                """Build step-(i+1) W tiles for feature chunk fi from zsrc."""
                for g in range(G):
                    if hn > 0:
                        nc.vector.tensor_tensor(
                            W16n[:, g, fi, :hn], th[:, g, :hn],
                            zsrc[:, fi, :hn], MULT)
                    if widn > 0:
                        gp, j = divmod(g, 2)
                        eng = nc.gpsimd if g >= G - POOL_W8_OPS else nc.vector
                        eng.tensor_tensor(
                            W8tn[:, gp, fi, j, hn:hn + widn],
                            th[:, g, hn:hn + widn],
                            zsrc[:, fi, hn:hn + widn], MULT)

            # ---- PE warmup: junk matmuls while the DMA stream lands ----
            for r in range(20):
                nc.tensor.matmul(work[3][:], xT[:, 0, :P], xT[:, r % FT, :],
                                 start=(r == 0), stop=(r == 19),
                                 skip_group_check=True)

            # ---- step-1 W from x ----
            n1, h1 = sched[0]
            W16 = w16_alloc(0) if h1 > 0 else None
            W8t = w8_alloc(0) if h1 < n1 else None
            for fi in range(FT):
                build_w_chunk(W16, W8t, xT, fi, h1, n1 - h1)

            deferred = []   # PE instructions delayed by one bank

            def flush_deferred():
                for fn in deferred:
                    fn()
                deferred.clear()

            for i in range(1, d + 1):
                n, h = sched[i - 1]
                wid = n - h
                nn, hn = sched[i] if i < d else (0, 0)
                widn = nn - hn
                W16n = w16_alloc(i) if hn > 0 else None
                W8tn = w8_alloc(i) if widn > 0 else None
                ztn = zpool.tile([P, FT, BLOC], F16, tag="z", name=f"z_{i}") \
                    if i < d else None

                last = (i == d)
                for ho in range(FT):
                    bank = work[ho] if not last else resb[ho]
                    first = not last
                    # fp16 group (fi-outer so next step's first k-tiles are
                    # ready right after this step's first drain)
                    if h > 0:
                        korder = [(g, fi) for fi in range(FT)
                                  for g in range(G)] if i > 1 else \
                                 [(g, fi) for g in range(G)
                                  for fi in range(FT)]
                        for nk, (g, fi) in enumerate(korder):
                            nc.tensor.matmul(
                                bank[:, :h],
                                A16[:, ho, g, fi, :],
                                W16[:, g, fi, :h],
                                start=first,
                                stop=(nk == NK - 1 and wid == 0),
                                skip_group_check=True)
                            first = False
                    if wid > 0:
                        for fi in range(FT):
                            for gp in range(GP):
                                nk = fi * GP + gp
                                nc.tensor.matmul(
                                    bank[:, h:n],
                                    A8[:, ho, gp, fi, :, :],
                                    W8t[:, gp, fi, :, h:n],
                                    start=first,
                                    stop=(nk == FT * GP - 1),
                                    perf_mode=DR,
                                    skip_group_check=True)
                                first = False

                    flush_deferred()

                    if i < d:
                        # drain bank ho: zt_i = psum / (i+1)  (fp16)
                        nc.scalar.mul(ztn[:, ho, :n], bank[:, :n],
                                      1.0 / (i + 1))
                        build_w_chunk(W16n, W8tn, ztn, ho, hn, widn)

                        def res_mm(i=i, ho=ho, n=n, ztn=ztn):
                            if i == 1:
                                nc.tensor.matmul(
                                    resb[ho][:], I16[:, 0, :], xT[:, ho, :],
                                    start=True, stop=False,
                                    skip_group_check=True)
                            nc.tensor.matmul(
                                resb[ho][:, :n], I16[:, i, :],
                                ztn[:, ho, :n], start=False, stop=False,
                                skip_group_check=True)
                        deferred.append(res_mm)
                    else:
                        ot = cpool.tile([P, BLOC], F32, tag=f"out{ho}",
                                        name=f"out_{ho}")
                        nc.scalar.mul(ot[:], resb[ho][:], 1.0)
                        nc.sync.dma_start(
                            out_d[ho * P:(ho + 1) * P, :], ot[:])

                W16, W8t, zt = W16n, W8tn, ztn
            flush_deferred()

    nc.compile()
    _cache[order] = nc
    return nc


def _in_maps(theta, x, algebra, order):
    sched = _sched(order)
    d = len(sched)
    use8 = any(h < n for (n, h) in sched)
    theta = np.ascontiguousarray(theta, dtype=np.float32)
    x = np.ascontiguousarray(x, dtype=np.float32)
    algebra = np.ascontiguousarray(algebra, dtype=np.float32)

    # per-core sort by Theta desc
    Th = (theta ** 2).sum(0)
    perm = np.concatenate(
        [c * BLOC + np.argsort(-Th[c * BLOC:(c + 1) * BLOC])
         for c in range(NCORES)])
    theta_s = theta[:, perm]
    x_s = x[perm]

    A4 = algebra.reshape(G, FT, P, FT, P)          # [g, fi, p, ho, m]
    A16_host = np.ascontiguousarray(
        A4.transpose(2, 3, 0, 1, 4).reshape(P, FT * NK * P).astype(np.float16))
    maps_common = {"A16": A16_host}
    if use8:
        A8v = algebra.reshape(GP, 2, FT, P, FT, P)  # [gp, j, fi, p, ho, m]
        A8_host = np.ascontiguousarray(
            A8v.transpose(3, 4, 0, 2, 1, 5).reshape(P, FT * NK * P)
            .astype(ml_dtypes.float8_e4m3fn))
        maps_common["A8"] = A8_host
    I_host = np.zeros((P, d, P), np.float16)
    for c in range(d):
        I_host[:, c, :] = np.eye(P, dtype=np.float16) * (c + 1)
    maps_common["I"] = np.ascontiguousarray(I_host.reshape(P, d * P))

    maps = []
    for c in range(NCORES):
        th_loc = theta_s[:, c * BLOC:(c + 1) * BLOC]          # [G, BLOC]
        th_b = np.ascontiguousarray(
            np.broadcast_to(th_loc[None], (P, G, BLOC))
            .reshape(P, G * BLOC).astype(np.float16))
        xT = np.ascontiguousarray(
            x_s[c * BLOC:(c + 1) * BLOC, :].T.reshape(FT, P, BLOC)
            .transpose(1, 0, 2).reshape(P, FT * BLOC).astype(np.float16))
        m = {"th": th_b, "xT": xT}
        m.update(maps_common)
        maps.append(m)
    return maps, perm


def _run(theta, x, algebra, order, **kw):
    order = int(order)
    nc = _build(order)
    maps, perm = _in_maps(theta, x, algebra, order)
    res = run_bass_kernel_spmd(nc, maps, list(range(NCORES)), **kw)
    out = np.empty((B, F), dtype=np.float32)
    for c in range(NCORES):
        out[perm[c * BLOC:(c + 1) * BLOC]] = res.results[c]["outT"].T
    return out, res


def kernel(theta, x, algebra, order):
    if int(order) <= 0:
        return np.ascontiguousarray(x, dtype=np.float32).copy()
    out, _ = _run(theta, x, algebra, order)
    return out
